# revision 2
# baseline (speedup 1.0000x reference)
"""Trainium2 Bass kernel for nn_DensityLoss (retrieval kNN hinge loss).

Computes mean(relu(topk_smallest_dist(x_pred, x_target, k) - 1.0)).

Strategy (8 NeuronCores, SPMD):
  - Shard x_pred rows across the 8 cores (1024 rows each).
  - Host pre-transposes both point sets to [dim, n] layout and precomputes
    -||b||^2 so the device only runs matmuls + selection.
  - Each core computes m = 2*a.b - ||b||^2 = ||a||^2 - d^2 per (row, target)
    via TensorE (bf16 operands, fp32 PSUM accum; the -||b||^2 row is folded
    in with a K=1 matmul), casts to fp16 into SBUF (ScalarE), then reduces
    16384 targets -> 1024 strided chunk-maxima with an elementwise max fold
    tree on the DVE (fp16 2x mode).
  - Chunk maxima [rows, 1024] DMA back to host. Host picks the top-8 chunks
    per row (guaranteed to contain the true top-5 targets: a top-5 target's
    chunk-max ranks <= 5 exactly; fp16 rounding noise is absorbed by the
    3-slot margin), rescores the 8*16 = 128 candidate targets exactly in
    float64, takes top-k, applies the hinge, and averages.
"""

import numpy as np

N_CORES = 8
N_PRED = 8192
N_TGT = 16384
DIM = 128
ROWS_PER_CORE = N_PRED // N_CORES  # 1024
ROWTILES = ROWS_PER_CORE // 128    # 8
BANK = 512                         # fp32 PSUM bank, matmul max N
GROUP = 4                          # banks per PSUM tile
N_CHUNK = N_TGT // BANK            # 32 matmul chunks per rowtile
FOLD_TO = 1024                     # chunk-max vector length after fold tree
FOLD_S = N_TGT // FOLD_TO          # 16 targets per fold chunk
TOP_CHUNKS = 8
HINGE = 1.0

_CACHE = {}


def _build_nc():
    import concourse.bacc as bacc
    import concourse.bass as bass
    import concourse.mybir as mybir
    import concourse.tile as tile

    dt = mybir.dt
    nc = bacc.Bacc(
        "TRN2",
        target_bir_lowering=False,
        debug=False,
        num_devices=N_CORES,
    )
    a_t = nc.dram_tensor("a_t", [DIM, ROWS_PER_CORE], dt.bfloat16, kind="ExternalInput")
    b_t = nc.dram_tensor("b_t", [DIM, N_TGT], dt.bfloat16, kind="ExternalInput")
    nb2 = nc.dram_tensor("nb2", [1, N_TGT], dt.float16, kind="ExternalInput")
    cmx = nc.dram_tensor(
        "cmx", [ROWTILES, 128, FOLD_TO], dt.float16, kind="ExternalOutput"
    )

    with tile.TileContext(nc) as tc:
        with (
            tc.tile_pool(name="const", bufs=1) as cpool,
            tc.tile_pool(name="psum", bufs=2, space="PSUM") as ppool,
            tc.tile_pool(name="slab", bufs=2) as spool,
            tc.tile_pool(name="fold", bufs=1) as fpool,
        ):
            bt_sb = cpool.tile([DIM, N_TGT], dt.bfloat16)
            at_sb = cpool.tile([DIM, ROWS_PER_CORE], dt.bfloat16)
            nb2_sb = cpool.tile([1, N_TGT], dt.float16)
            ones_sb = cpool.tile([1, DIM], dt.float16)

            # Split the big input DMA so compute can start on early slices.
            n_dma = 8
            for s in range(n_dma):
                sl = bass.ts(s, N_TGT // n_dma)
                nc.sync.dma_start(out=bt_sb[:, sl], in_=b_t[:, sl])
            nc.sync.dma_start(out=at_sb[:], in_=a_t[:])
            nc.sync.dma_start(out=nb2_sb[:], in_=nb2[:])
            nc.gpsimd.memset(ones_sb[:], 1.0)

            for rt in range(ROWTILES):
                lhsT = at_sb[:, bass.ts(rt, 128)]
                slab = spool.tile([128, N_TGT], dt.float16)
                for g in range(N_CHUNK // GROUP):
                    ps = ppool.tile([128, BANK * GROUP], dt.float32)
                    for j in range(GROUP):
                        c = g * GROUP + j
                        nc.tensor.matmul(
                            ps[:, bass.ts(j, BANK)],
                            lhsT,
                            bt_sb[:, bass.ts(c, BANK)],
                            start=True,
                            stop=False,
                        )
                        nc.tensor.matmul(
                            ps[:, bass.ts(j, BANK)],
                            ones_sb[:],
                            nb2_sb[:, bass.ts(c, BANK)],
                            start=False,
                            stop=True,
                        )
                    nc.scalar.copy(slab[:, bass.ts(g, BANK * GROUP)], ps[:])
                f = slab
                w = N_TGT
                while w > FOLD_TO:
                    w //= 2
                    nf = fpool.tile([128, w], dt.float16, tag=f"f{w}")
                    nc.vector.tensor_max(nf[:], f[:, 0:w], f[:, w : 2 * w])
                    f = nf
                nc.sync.dma_start(out=cmx[rt], in_=f[:])

    nc.compile()
    return nc


def _get_nc():
    if "nc" not in _CACHE:
        _CACHE["nc"] = _build_nc()
    return _CACHE["nc"]


def _host_finish(x_pred, x_target, chunk_max, k):
    """chunk_max: [N_PRED, FOLD_TO] float32 of per-chunk maxima of
    m = 2 a.b - b2. Chunk j holds targets {j + FOLD_TO*i}."""
    n = x_pred.shape[0]
    ch = np.argpartition(-chunk_max, TOP_CHUNKS, axis=1)[:, :TOP_CHUNKS]
    tid = (
        ch[:, :, None] + FOLD_TO * np.arange(FOLD_S)[None, None, :]
    ).reshape(n, TOP_CHUNKS * FOLD_S)

    a64 = x_pred.astype(np.float64)
    b64 = x_target.astype(np.float64)
    a2 = np.einsum("ij,ij->i", a64, a64)
    b2 = np.einsum("ij,ij->i", b64, b64)

    vals = np.empty((n, k))
    B = 1024
    for s in range(0, n, B):
        t = tid[s : s + B]
        bg = b64[t]  # [B, C, DIM]
        dots = np.einsum("rd,rcd->rc", a64[s : s + B], bg, optimize=True)
        d2 = a2[s : s + B, None] + b2[t] - 2.0 * dots
        vals[s : s + B] = np.partition(d2, k - 1, axis=1)[:, :k]
    d = np.sqrt(np.maximum(vals, 0.0))
    return np.float32(np.maximum(d - HINGE, 0.0).mean(dtype=np.float64))


def _host_exact(x_pred, x_target, k):
    """Exact fallback (never expected in practice)."""
    a = x_pred.astype(np.float32)
    b = x_target.astype(np.float32)
    a2 = np.sum(a * a, axis=1)[:, None]
    b2 = np.sum(b * b, axis=1)[None, :]
    out = np.empty((a.shape[0], k), np.float64)
    B = 1024
    for s in range(0, a.shape[0], B):
        d2 = a2[s : s + B] + b2 - 2.0 * (a[s : s + B] @ b.T)
        out[s : s + B] = np.partition(d2, k - 1, axis=1)[:, :k].astype(np.float64)
    d = np.sqrt(np.maximum(out, 0.0))
    return np.float32(np.maximum(d - HINGE, 0.0).mean(dtype=np.float64))


def kernel(x_pred, x_target, top_k=5, _want_results=False):
    import ml_dtypes
    from concourse.bass_utils import run_bass_kernel_spmd

    x_pred = np.asarray(x_pred, dtype=np.float32)
    x_target = np.asarray(x_target, dtype=np.float32)
    k = int(top_k)
    if (
        k > TOP_CHUNKS
        or x_pred.shape != (N_PRED, DIM)
        or x_target.shape != (N_TGT, DIM)
    ):
        return _host_exact(x_pred, x_target, k)

    nc = _get_nc()

    # Factor 2 of the cross term 2*a.b is folded into a (exact in bf16).
    a_t_full = np.ascontiguousarray(2.0 * x_pred.T).astype(ml_dtypes.bfloat16)
    b_t = np.ascontiguousarray(x_target.T).astype(ml_dtypes.bfloat16)
    b2 = np.einsum("ij,ij->i", x_target, x_target, dtype=np.float64)
    nb2 = (-b2).astype(np.float16)[None, :]

    in_maps = []
    for c in range(N_CORES):
        in_maps.append(
            {
                "a_t": np.ascontiguousarray(
                    a_t_full[:, c * ROWS_PER_CORE : (c + 1) * ROWS_PER_CORE]
                ),
                "b_t": b_t,
                "nb2": nb2,
            }
        )

    res = run_bass_kernel_spmd(nc, in_maps, list(range(N_CORES)))
    chunk_max = np.concatenate(
        [
            res.results[c]["cmx"].reshape(ROWS_PER_CORE, FOLD_TO)
            for c in range(N_CORES)
        ],
        axis=0,
    ).astype(np.float32)
    out = _host_finish(x_pred, x_target, chunk_max, k)
    if _want_results:
        return out, res
    return out


# revision 3
# speedup vs baseline: 1.1497x; 1.1497x over previous
"""Trainium2 Bass kernel for nn_DensityLoss (retrieval kNN hinge loss).

Computes mean(relu(topk_smallest_dist(x_pred, x_target, k) - 1.0)).

Strategy (8 NeuronCores, SPMD):
  - Shard x_pred rows across the 8 cores (1024 rows each).
  - Host pre-transposes both point sets to [dim, n] layout and precomputes
    -||b||^2 so the device only runs matmuls + selection.
  - Each core computes m = 2*a.b - ||b||^2 = ||a||^2 - d^2 per (row, target)
    via TensorE (bf16 operands, fp32 PSUM accum; the -||b||^2 row is folded
    in with a K=1 matmul), casts to fp16 into SBUF (ScalarE), then reduces
    16384 targets -> 1024 strided chunk-maxima with an elementwise max fold
    tree on the DVE (fp16 2x mode).
  - Chunk maxima [rows, 1024] DMA back to host. Host picks the top-8 chunks
    per row (guaranteed to contain the true top-5 targets: a top-5 target's
    chunk-max ranks <= 5 exactly; fp16 rounding noise is absorbed by the
    3-slot margin), rescores the 8*16 = 128 candidate targets exactly in
    float64, takes top-k, applies the hinge, and averages.
"""

import numpy as np

N_CORES = 8
N_PRED = 8192
N_TGT = 16384
DIM = 128
ROWS_PER_CORE = N_PRED // N_CORES  # 1024
ROWTILES = ROWS_PER_CORE // 128    # 8
BANK = 512                         # fp32 PSUM bank, matmul max N
GROUP = 4                          # banks per PSUM tile
N_CHUNK = N_TGT // BANK            # 32 matmul chunks per rowtile
FOLD_TO = 1024                     # chunk-max vector length after fold tree
FOLD_S = N_TGT // FOLD_TO          # 16 targets per fold chunk
TOP_CHUNKS = 8
HINGE = 1.0

_CACHE = {}


def _build_nc():
    import concourse.bacc as bacc
    import concourse.bass as bass
    import concourse.mybir as mybir
    import concourse.tile as tile

    dt = mybir.dt
    nc = bacc.Bacc(
        "TRN2",
        target_bir_lowering=False,
        debug=False,
        num_devices=N_CORES,
    )
    a_t = nc.dram_tensor("a_t", [DIM, ROWS_PER_CORE], dt.bfloat16, kind="ExternalInput")
    b_t = nc.dram_tensor("b_t", [DIM, N_TGT], dt.bfloat16, kind="ExternalInput")
    nb2 = nc.dram_tensor("nb2", [1, N_TGT], dt.float16, kind="ExternalInput")
    cmx = nc.dram_tensor(
        "cmx", [ROWTILES, 128, FOLD_TO], dt.float16, kind="ExternalOutput"
    )

    with tile.TileContext(nc) as tc:
        with (
            tc.tile_pool(name="const", bufs=1) as cpool,
            tc.tile_pool(name="psum", bufs=2, space="PSUM") as ppool,
            tc.tile_pool(name="slab", bufs=2) as spool,
            tc.tile_pool(name="fold", bufs=1) as fpool,
        ):
            bt_sb = cpool.tile([DIM, N_TGT], dt.bfloat16)
            at_sb = cpool.tile([DIM, ROWS_PER_CORE], dt.bfloat16)
            nb2_sb = cpool.tile([1, N_TGT], dt.float16)
            ones_sb = cpool.tile([1, DIM], dt.float16)

            # Split the big input DMA so compute can start on early slices.
            n_dma = 8
            for s in range(n_dma):
                sl = bass.ts(s, N_TGT // n_dma)
                nc.sync.dma_start(out=bt_sb[:, sl], in_=b_t[:, sl])
            nc.sync.dma_start(out=at_sb[:], in_=a_t[:])
            nc.sync.dma_start(out=nb2_sb[:], in_=nb2[:])
            nc.gpsimd.memset(ones_sb[:], 1.0)

            for rt in range(ROWTILES):
                lhsT = at_sb[:, bass.ts(rt, 128)]
                slab = spool.tile([128, N_TGT], dt.float16)
                for g in range(N_CHUNK // GROUP):
                    ps = ppool.tile([128, BANK * GROUP], dt.float32)
                    # Batch by stationary operand so the PE streams
                    # back-to-back instead of reloading weights per matmul.
                    for j in range(GROUP):
                        c = g * GROUP + j
                        nc.tensor.matmul(
                            ps[:, bass.ts(j, BANK)],
                            lhsT,
                            bt_sb[:, bass.ts(c, BANK)],
                            start=True,
                            stop=False,
                        )
                    for j in range(GROUP):
                        c = g * GROUP + j
                        nc.tensor.matmul(
                            ps[:, bass.ts(j, BANK)],
                            ones_sb[:],
                            nb2_sb[:, bass.ts(c, BANK)],
                            start=False,
                            stop=True,
                        )
                    nc.scalar.copy(slab[:, bass.ts(g, BANK * GROUP)], ps[:])
                f = slab
                w = N_TGT
                while w > FOLD_TO:
                    w //= 2
                    nf = fpool.tile([128, w], dt.float16, tag=f"f{w}")
                    nc.vector.tensor_max(nf[:], f[:, 0:w], f[:, w : 2 * w])
                    f = nf
                nc.sync.dma_start(out=cmx[rt], in_=f[:])

    nc.compile()
    return nc


def _get_nc():
    if "nc" not in _CACHE:
        _CACHE["nc"] = _build_nc()
    return _CACHE["nc"]


def _host_finish(x_pred, x_target, chunk_max, k):
    """chunk_max: [N_PRED, FOLD_TO] float32 of per-chunk maxima of
    m = 2 a.b - b2. Chunk j holds targets {j + FOLD_TO*i}."""
    n = x_pred.shape[0]
    ch = np.argpartition(-chunk_max, TOP_CHUNKS, axis=1)[:, :TOP_CHUNKS]
    tid = (
        ch[:, :, None] + FOLD_TO * np.arange(FOLD_S)[None, None, :]
    ).reshape(n, TOP_CHUNKS * FOLD_S)

    a64 = x_pred.astype(np.float64)
    b64 = x_target.astype(np.float64)
    a2 = np.einsum("ij,ij->i", a64, a64)
    b2 = np.einsum("ij,ij->i", b64, b64)

    vals = np.empty((n, k))
    B = 1024
    for s in range(0, n, B):
        t = tid[s : s + B]
        bg = b64[t]  # [B, C, DIM]
        dots = np.einsum("rd,rcd->rc", a64[s : s + B], bg, optimize=True)
        d2 = a2[s : s + B, None] + b2[t] - 2.0 * dots
        vals[s : s + B] = np.partition(d2, k - 1, axis=1)[:, :k]
    d = np.sqrt(np.maximum(vals, 0.0))
    return np.float32(np.maximum(d - HINGE, 0.0).mean(dtype=np.float64))


def _host_exact(x_pred, x_target, k):
    """Exact fallback (never expected in practice)."""
    a = x_pred.astype(np.float32)
    b = x_target.astype(np.float32)
    a2 = np.sum(a * a, axis=1)[:, None]
    b2 = np.sum(b * b, axis=1)[None, :]
    out = np.empty((a.shape[0], k), np.float64)
    B = 1024
    for s in range(0, a.shape[0], B):
        d2 = a2[s : s + B] + b2 - 2.0 * (a[s : s + B] @ b.T)
        out[s : s + B] = np.partition(d2, k - 1, axis=1)[:, :k].astype(np.float64)
    d = np.sqrt(np.maximum(out, 0.0))
    return np.float32(np.maximum(d - HINGE, 0.0).mean(dtype=np.float64))


def kernel(x_pred, x_target, top_k=5, _want_results=False):
    import ml_dtypes
    from concourse.bass_utils import run_bass_kernel_spmd

    x_pred = np.asarray(x_pred, dtype=np.float32)
    x_target = np.asarray(x_target, dtype=np.float32)
    k = int(top_k)
    if (
        k > TOP_CHUNKS
        or x_pred.shape != (N_PRED, DIM)
        or x_target.shape != (N_TGT, DIM)
    ):
        return _host_exact(x_pred, x_target, k)

    nc = _get_nc()

    # Factor 2 of the cross term 2*a.b is folded into a (exact in bf16).
    a_t_full = np.ascontiguousarray(2.0 * x_pred.T).astype(ml_dtypes.bfloat16)
    b_t = np.ascontiguousarray(x_target.T).astype(ml_dtypes.bfloat16)
    b2 = np.einsum("ij,ij->i", x_target, x_target, dtype=np.float64)
    nb2 = (-b2).astype(np.float16)[None, :]

    in_maps = []
    for c in range(N_CORES):
        in_maps.append(
            {
                "a_t": np.ascontiguousarray(
                    a_t_full[:, c * ROWS_PER_CORE : (c + 1) * ROWS_PER_CORE]
                ),
                "b_t": b_t,
                "nb2": nb2,
            }
        )

    res = run_bass_kernel_spmd(nc, in_maps, list(range(N_CORES)))
    chunk_max = np.concatenate(
        [
            res.results[c]["cmx"].reshape(ROWS_PER_CORE, FOLD_TO)
            for c in range(N_CORES)
        ],
        axis=0,
    ).astype(np.float32)
    out = _host_finish(x_pred, x_target, chunk_max, k)
    if _want_results:
        return out, res
    return out


# revision 4
# speedup vs baseline: 1.6637x; 1.4471x over previous
"""Trainium2 Bass kernel for nn_DensityLoss (retrieval kNN hinge loss).

Computes mean(relu(topk_smallest_dist(x_pred, x_target, k) - 1.0)).

Strategy (8 NeuronCores, SPMD):
  - Shard x_pred rows across the 8 cores (1024 rows each).
  - Host pre-transposes both point sets to [dim, n] layout (factor 2 of the
    cross term folded into a), and precomputes -||b||^2 replicated across
    the 128 partitions.
  - Each core computes 2*a.b via TensorE (bf16 operands, fp32 PSUM accum),
    ScalarE casts PSUM to fp16 into an SBUF slab, DVE adds -||b||^2
    (fp16 2x mode, in place) giving m = 2 a.b - b2 = ||a||^2 - d^2, then an
    elementwise-max fold tree on DVE reduces 16384 targets -> 1024 strided
    chunk-maxima per row.
  - Chunk maxima [rows, 1024] DMA back to host. Host picks the top-8 chunks
    per row (guaranteed to contain the true top-5 targets: a top-5 target's
    chunk-max ranks <= 5 exactly; fp16 rounding noise is absorbed by the
    3-slot margin), rescores the 8*16 = 128 candidate targets exactly in
    float64, takes top-k, applies the hinge, and averages.
"""

import numpy as np

N_CORES = 8
N_PRED = 8192
N_TGT = 16384
DIM = 128
ROWS_PER_CORE = N_PRED // N_CORES  # 1024
ROWTILES = ROWS_PER_CORE // 128    # 8
BANK = 512                         # fp32 PSUM bank, matmul max N
GROUP = 4                          # banks per PSUM tile
N_CHUNK = N_TGT // BANK            # 32 matmul chunks per rowtile
FOLD_TO = 1024                     # chunk-max vector length after fold tree
FOLD_S = N_TGT // FOLD_TO          # 16 targets per fold chunk
TOP_CHUNKS = 8
HINGE = 1.0

_CACHE = {}


def _build_nc():
    import concourse.bacc as bacc
    import concourse.bass as bass
    import concourse.mybir as mybir
    import concourse.tile as tile

    dt = mybir.dt
    nc = bacc.Bacc(
        "TRN2",
        target_bir_lowering=False,
        debug=False,
        num_devices=N_CORES,
    )
    a_t = nc.dram_tensor("a_t", [DIM, ROWS_PER_CORE], dt.bfloat16, kind="ExternalInput")
    b_t = nc.dram_tensor("b_t", [DIM, N_TGT], dt.bfloat16, kind="ExternalInput")
    nb2 = nc.dram_tensor("nb2", [128, N_TGT], dt.float16, kind="ExternalInput")
    cmx = nc.dram_tensor(
        "cmx", [ROWTILES, 128, FOLD_TO], dt.float16, kind="ExternalOutput"
    )

    with tile.TileContext(nc) as tc:
        with (
            tc.tile_pool(name="const", bufs=1) as cpool,
            tc.tile_pool(name="psum", bufs=2, space="PSUM") as ppool,
            tc.tile_pool(name="slab", bufs=2) as spool,
            tc.tile_pool(name="fold", bufs=1) as fpool,
        ):
            bt_sb = cpool.tile([DIM, N_TGT], dt.bfloat16)
            at_sb = cpool.tile([DIM, ROWS_PER_CORE], dt.bfloat16)
            nb2_sb = cpool.tile([128, N_TGT], dt.float16)

            # Split the big input DMAs so compute can start on early slices.
            n_dma = 8
            for s in range(n_dma):
                sl = bass.ts(s, N_TGT // n_dma)
                nc.sync.dma_start(out=bt_sb[:, sl], in_=b_t[:, sl])
                nc.sync.dma_start(out=nb2_sb[:, sl], in_=nb2[:, sl])
            nc.sync.dma_start(out=at_sb[:], in_=a_t[:])

            for rt in range(ROWTILES):
                lhsT = at_sb[:, bass.ts(rt, 128)]
                slab = spool.tile([128, N_TGT], dt.float16)
                for g in range(N_CHUNK // GROUP):
                    ps = ppool.tile([128, BANK * GROUP], dt.float32)
                    for j in range(GROUP):
                        c = g * GROUP + j
                        nc.tensor.matmul(
                            ps[:, bass.ts(j, BANK)],
                            lhsT,
                            bt_sb[:, bass.ts(c, BANK)],
                            start=True,
                            stop=True,
                        )
                    gsl = bass.ts(g, BANK * GROUP)
                    nc.scalar.copy(slab[:, gsl], ps[:])
                    # m = 2 a.b - b2, in place (fp16, DVE 2x mode)
                    nc.vector.tensor_add(slab[:, gsl], slab[:, gsl], nb2_sb[:, gsl])
                f = slab
                w = N_TGT
                while w > FOLD_TO:
                    w //= 2
                    nf = fpool.tile([128, w], dt.float16, tag=f"f{w}")
                    nc.vector.tensor_max(nf[:], f[:, 0:w], f[:, w : 2 * w])
                    f = nf
                nc.sync.dma_start(out=cmx[rt], in_=f[:])

    nc.compile()
    return nc


def _get_nc():
    if "nc" not in _CACHE:
        _CACHE["nc"] = _build_nc()
    return _CACHE["nc"]


def _host_finish(x_pred, x_target, chunk_max, k):
    """chunk_max: [N_PRED, FOLD_TO] float32 of per-chunk maxima of
    m = 2 a.b - b2. Chunk j holds targets {j + FOLD_TO*i}."""
    n = x_pred.shape[0]
    ch = np.argpartition(-chunk_max, TOP_CHUNKS, axis=1)[:, :TOP_CHUNKS]
    tid = (
        ch[:, :, None] + FOLD_TO * np.arange(FOLD_S)[None, None, :]
    ).reshape(n, TOP_CHUNKS * FOLD_S)

    a64 = x_pred.astype(np.float64)
    b64 = x_target.astype(np.float64)
    a2 = np.einsum("ij,ij->i", a64, a64)
    b2 = np.einsum("ij,ij->i", b64, b64)

    vals = np.empty((n, k))
    B = 1024
    for s in range(0, n, B):
        t = tid[s : s + B]
        bg = b64[t]  # [B, C, DIM]
        dots = np.einsum("rd,rcd->rc", a64[s : s + B], bg, optimize=True)
        d2 = a2[s : s + B, None] + b2[t] - 2.0 * dots
        vals[s : s + B] = np.partition(d2, k - 1, axis=1)[:, :k]
    d = np.sqrt(np.maximum(vals, 0.0))
    return np.float32(np.maximum(d - HINGE, 0.0).mean(dtype=np.float64))


def _host_exact(x_pred, x_target, k):
    """Exact fallback (never expected in practice)."""
    a = x_pred.astype(np.float32)
    b = x_target.astype(np.float32)
    a2 = np.sum(a * a, axis=1)[:, None]
    b2 = np.sum(b * b, axis=1)[None, :]
    out = np.empty((a.shape[0], k), np.float64)
    B = 1024
    for s in range(0, a.shape[0], B):
        d2 = a2[s : s + B] + b2 - 2.0 * (a[s : s + B] @ b.T)
        out[s : s + B] = np.partition(d2, k - 1, axis=1)[:, :k].astype(np.float64)
    d = np.sqrt(np.maximum(out, 0.0))
    return np.float32(np.maximum(d - HINGE, 0.0).mean(dtype=np.float64))


def kernel(x_pred, x_target, top_k=5, _want_results=False):
    import ml_dtypes
    from concourse.bass_utils import run_bass_kernel_spmd

    x_pred = np.asarray(x_pred, dtype=np.float32)
    x_target = np.asarray(x_target, dtype=np.float32)
    k = int(top_k)
    if (
        k > TOP_CHUNKS
        or x_pred.shape != (N_PRED, DIM)
        or x_target.shape != (N_TGT, DIM)
    ):
        return _host_exact(x_pred, x_target, k)

    nc = _get_nc()

    # Factor 2 of the cross term 2*a.b is folded into a (exact in bf16).
    a_t_full = np.ascontiguousarray(2.0 * x_pred.T).astype(ml_dtypes.bfloat16)
    b_t = np.ascontiguousarray(x_target.T).astype(ml_dtypes.bfloat16)
    b2 = np.einsum("ij,ij->i", x_target, x_target, dtype=np.float64)
    nb2 = np.broadcast_to((-b2).astype(np.float16)[None, :], (128, N_TGT))
    nb2 = np.ascontiguousarray(nb2)

    in_maps = []
    for c in range(N_CORES):
        in_maps.append(
            {
                "a_t": np.ascontiguousarray(
                    a_t_full[:, c * ROWS_PER_CORE : (c + 1) * ROWS_PER_CORE]
                ),
                "b_t": b_t,
                "nb2": nb2,
            }
        )

    res = run_bass_kernel_spmd(nc, in_maps, list(range(N_CORES)))
    chunk_max = np.concatenate(
        [
            res.results[c]["cmx"].reshape(ROWS_PER_CORE, FOLD_TO)
            for c in range(N_CORES)
        ],
        axis=0,
    ).astype(np.float32)
    out = _host_finish(x_pred, x_target, chunk_max, k)
    if _want_results:
        return out, res
    return out
